# revision 17
# baseline (speedup 1.0000x reference)
"""Trainium2 (8-core SPMD) kernel for the ActorCriticTensorNet MPS head.

reference:
    env0 = einsum('e,eoij->oij', x[0], mps[0])
    for a in 1..63: env = sigmoid(env @ einsum('e,eoij->oij', x[a], mps[a]))
    out = einsum('oii->o', env)

Strategy: the computation factorizes perfectly over the output channel o —
the per-agent contractions mat[a][o] and the 63-step sigmoid chain for
channel o never touch any other channel; the channels only meet in the final
trace vector.  So shard by o: core c receives mps[:, :, c, :, :] (16.8 MB in
fp8) plus the full (tiny) x, computes all 64 mat[g][c] slices locally, runs
its own o=c chain locally, and ships its final 32x32 env; the host takes the
8 traces.  Zero inter-core communication, and the chain consumes mats as
phase 1 produces them, so its serial latency hides under the tensor
streaming.

Phase-1 layout (per agent block): weight column (gi, c) is the contiguous
run packed[gi*1024 + c*128 : +128] with per-column packing p = 32*jh + k
(k = chain row, jh = j>>3, c = j&7), so psum[32*jh + k, gi*8 + c] =
mat[gi][k, 8*jh + c].  Four 32-partition DVE copies (cross-quadrant moves
are free) then drop each psum quadrant jh straight into the chain-weight
slab cw[k, gi*32 + 8*jh + c] = mat[gi][k, j] in SBUF — no DRAM round trip,
no relayout DMAs.

Phase-1 operands are fp8 e3m4 with power-of-two scales (mps x128, x x2 —
both mid-range in e3m4's +-15.5), halving HBM traffic vs bf16 and enabling
4x fast-weight-load.  PSUM accumulates the 256x-scaled mats in fp32; the
1/256 descale folds into the chain sigmoid's scale parameter.  The chain
runs in bf16 (simulated end-to-end relative error ~1.4e-3; gate is 2e-2).
"""

import numpy as np

A, E, O, C = 64, 256, 8, 32
FO = C * C  # per-o mat size: 1024
N_CORES = 8
# phase-1/chain segments (start_agent, n_agents): small at the head so the
# PE starts early, small at the tail so the final chain steps hand off at
# fine granularity, big in the middle for DMA efficiency.
SEGS = [(4 * b, 4) for b in range(14)] + [(56, 2), (58, 2), (60, 2), (62, 2)]

_CACHE = {}


def _build():
    from concourse import bacc, mybir, tile
    from concourse.masks import make_identity
    from concourse.tile_rust import add_dep_helper

    F32 = mybir.dt.float32
    BF16 = mybir.dt.bfloat16
    FP8 = mybir.dt.float8e3
    SIG = mybir.ActivationFunctionType.Sigmoid
    COPY = mybir.ActivationFunctionType.Copy
    nc = bacc.Bacc(
        "TRN2", target_bir_lowering=False, debug=False, num_devices=N_CORES
    )
    x_d = nc.dram_tensor("inputs", [2, 128, A], FP8, kind="ExternalInput")
    mps_d = nc.dram_tensor(
        "mps", [2, 128, A * FO], FP8, kind="ExternalInput"
    )
    out_d = nc.dram_tensor("out", [C, C], BF16, kind="ExternalOutput")

    with tile.TileContext(nc) as tc:
        with (
            tc.tile_pool(name="mps_pool", bufs=9) as mps_pool,
            tc.tile_pool(name="small", bufs=1) as small,
            tc.tile_pool(name="cw_pool", bufs=6) as cw_pool,
            tc.tile_pool(name="env_pool", bufs=4) as env_pool,
            tc.tile_pool(name="ps_mat", bufs=4, space="PSUM") as ps_mat,
            tc.tile_pool(name="ps_chain", bufs=3, space="PSUM") as ps_chain,
        ):
            seg_of = {}
            for si, (g0, w) in enumerate(SEGS):
                for g in range(g0, g0 + w):
                    seg_of[g] = si

            # x_sb[e_lo, eh*64 + g] = x[g, eh*128 + e_lo]
            x_sb = small.tile([128, 2 * A], FP8)
            for eh in range(2):
                nc.sync.dma_start(x_sb[:, eh * A : (eh + 1) * A], x_d[eh])

            ident = small.tile([C, C], BF16)
            make_identity(nc, ident[:])

            envs = [None]
            cvs = {}

            def chain_step(g):
                si = seg_of[g]
                cv, gi = cvs[si], g - SEGS[si][0]
                init = g == 0
                ps_g = ps_chain.tile([C, C], BF16 if init else F32, tag="cps")
                if init:
                    pe = nc.tensor.transpose(ps_g[:], cv[:, gi, :], ident[:])
                else:
                    pe = nc.tensor.matmul(
                        ps_g[:],
                        cv[:, gi, :],
                        envs[0][:],
                        start=True,
                        stop=True,
                    )
                env2 = env_pool.tile([C, C], BF16, tag="env")
                if init:
                    # cw holds 256*mat (fp8 input scales 128*2); descale
                    nc.scalar.activation(
                        env2[:], ps_g[:], COPY, scale=1.0 / 256.0
                    )
                else:
                    # psum = env @ (256*mat); sigmoid's scale descales it
                    nc.scalar.activation(
                        env2[:], ps_g[:], SIG, scale=1.0 / 256.0
                    )
                envs[0] = env2
                return pe

            # Software-pipelined emission: the chain lags phase 1 by two
            # segments (so the DVE relayout of its weights has retired) and
            # is paced at one step per agent-slot (two when backlogged).
            # add_dep_helper pins each agent's phase-1 block AFTER the
            # latest chain matmul in the PE stream, so the Tile scheduler
            # cannot batch a segment's matmuls ahead of the chain — the
            # previous sigmoid retires while the phase-1 block runs and the
            # chain never head-of-line-blocks the PE.
            next_chain = 0
            last_chain = [None]

            for si, (g0, w) in enumerate(SEGS):
                psum_b = ps_mat.tile([128, 8 * w], F32, tag="psa")
                tvs = []
                for eh in range(2):
                    t = mps_pool.tile([128, w * FO], FP8, tag="mps")
                    nc.sync.dma_start(
                        t[:], mps_d[eh, :, g0 * FO : (g0 + w) * FO]
                    )
                    tvs.append(
                        t[:].rearrange(
                            "e (gi c p) -> e gi c p", gi=w, c=8, p=128
                        )
                    )
                for gi in range(w):
                    g = g0 + gi
                    for c in range(8):
                        for eh in range(2):
                            mm = nc.tensor.matmul(
                                psum_b[:, gi * 8 + c : gi * 8 + c + 1],
                                tvs[eh][:, gi, c, :],
                                x_sb[:, eh * A + g : eh * A + g + 1],
                                start=(eh == 0),
                                stop=(eh == 1),
                            )
                            if last_chain[0] is not None:
                                # pin EVERY phase-1 matmul after the latest
                                # chain matmul (sync=True: no_sync edges are
                                # ignored for ordering): the scheduler
                                # cannot run phase-1 ahead, so the PE stream
                                # strictly alternates [agent block][chain
                                # step] and each sigmoid retires while the
                                # next agent block streams
                                add_dep_helper(
                                    mm.ins,
                                    last_chain[0].ins,
                                    sync=True,
                                    reason="interleave chain with phase-1",
                                )
                    # agents eligible for the chain: segments <= si - 2
                    elig = SEGS[si - 1][0] if si >= 1 else 0
                    backlog = elig - next_chain
                    cap = 2 if backlog > 4 else 1
                    k = 0
                    while next_chain < elig and k < cap:
                        last_chain[0] = chain_step(next_chain)
                        next_chain += 1
                        k += 1
                # psum[32*jh + k, gi*8 + c] -> cw[k, gi*32 + jh*8 + c]:
                # one cross-quadrant DVE copy per psum quadrant jh.
                cw = cw_pool.tile([C, 32 * w], BF16, tag="cw", name=f"cw{si}")
                cwv = cw[:].rearrange(
                    "k (gi jh c) -> k gi jh c", gi=w, jh=4, c=8
                )
                for jh in range(4):
                    nc.vector.tensor_copy(
                        cwv[:, :, jh, :],
                        psum_b[32 * jh : 32 * jh + 32, :].rearrange(
                            "k (gi c) -> k gi c", gi=w, c=8
                        ),
                    )
                cvs[si] = cw[:].rearrange("k (gi j) -> k gi j", gi=w, j=32)
            while next_chain < A:
                chain_step(next_chain)
                next_chain += 1

            # ship the final 32x32 env; host takes the trace
            nc.sync.dma_start(out_d[:], envs[0][:])

    nc.compile()
    return nc


def get_nc():
    if "nc" not in _CACHE:
        _CACHE["nc"] = _build()
    return _CACHE["nc"]


def make_in_maps(inputs, mps):
    import ml_dtypes

    FP8 = ml_dtypes.float8_e3m4
    # power-of-two scales put both operands mid-range in e3m4 (max 15.5):
    # x ~ N(0,1) * 2, mps ~ N(0, 0.0156^2) * 128.  Combined 256x descales
    # on-device via the chain sigmoid's scale parameter.
    x = (np.asarray(inputs, dtype=np.float32) * 2.0).astype(FP8)
    mps = np.asarray(mps, dtype=np.float32).reshape(A, E, O, FO)
    # x packed as [e_chunk, e_low, agent]
    x_pack = np.ascontiguousarray(x.reshape(A, 2, 128).transpose(1, 2, 0))
    # F_idx[c, p] = k*32 + j with k = p%32, j = 8*(p//32) + c: weight column
    # (gi, c) reads the contiguous run packed[gi*1024 + c*128 : +128] and
    # psum partitions come out as 32*jh + k (chain-quadrant layout).
    p = np.arange(128)
    c = np.arange(8)[:, None]
    F_idx = ((p % 32) * 32 + (p // 32) * 8 + c).reshape(-1)  # (1024,)
    in_maps = []
    for ci in range(N_CORES):
        m = (mps[:, :, ci, :] * 128.0).astype(FP8)  # (A, E, FO)
        m = m[:, :, F_idx]  # permute f so weight columns are contiguous
        m = m.reshape(A, 2, 128, FO).transpose(1, 2, 0, 3)  # (2, 128, A, FO)
        in_maps.append(
            {
                "inputs": x_pack,
                "mps": np.ascontiguousarray(m).reshape(2, 128, A * FO),
            }
        )
    return in_maps


def kernel(inputs, mps):
    from concourse.bass_utils import run_bass_kernel_spmd

    nc = get_nc()
    in_maps = make_in_maps(inputs, mps)
    try:
        res = run_bass_kernel_spmd(nc, in_maps, core_ids=list(range(N_CORES)))
    except Exception:
        # rare transient NRT failures; one retry
        res = run_bass_kernel_spmd(nc, in_maps, core_ids=list(range(N_CORES)))
    return np.array(
        [
            np.trace(res.results[ci]["out"].astype(np.float32))
            for ci in range(N_CORES)
        ],
        dtype=np.float32,
    )


# revision 19
# speedup vs baseline: 1.3550x; 1.3550x over previous
"""Trainium2 (8-core SPMD) kernel for the ActorCriticTensorNet MPS head.

reference:
    env0 = einsum('e,eoij->oij', x[0], mps[0])
    for a in 1..63: env = sigmoid(env @ einsum('e,eoij->oij', x[a], mps[a]))
    out = einsum('oii->o', env)

Strategy: the computation factorizes perfectly over the output channel o —
the per-agent contractions mat[a][o] and the 63-step sigmoid chain for
channel o never touch any other channel; the channels only meet in the final
trace vector.  So shard by o: core c receives mps[:, :, c, :, :] (16.8 MB in
fp8) plus the full (tiny) x, computes all 64 mat[g][c] slices locally, runs
its own o=c chain locally, and ships its final 32x32 env; the host takes the
8 traces.  Zero inter-core communication, and the chain consumes mats as
phase 1 produces them, so its serial latency hides under the tensor
streaming.

Phase-1 layout (per agent block): weight column (gi, c) is the contiguous
run packed[gi*1024 + c*128 : +128] with per-column packing p = 32*jh + k
(k = chain row, jh = j>>3, c = j&7), so psum[32*jh + k, gi*8 + c] =
mat[gi][k, 8*jh + c].  Four 32-partition DVE copies (cross-quadrant moves
are free) then drop each psum quadrant jh straight into the chain-weight
slab cw[k, gi*32 + 8*jh + c] = mat[gi][k, j] in SBUF — no DRAM round trip,
no relayout DMAs.

Phase-1 operands are fp8 e3m4 with power-of-two scales (mps x128, x x2 —
both mid-range in e3m4's +-15.5), halving HBM traffic vs bf16 and enabling
4x fast-weight-load.  PSUM accumulates the 256x-scaled mats in fp32; the
1/256 descale folds into the chain sigmoid's scale parameter.  The chain
runs in bf16 (simulated end-to-end relative error ~1.4e-3; gate is 2e-2).
"""

import numpy as np

A, E, O, C = 64, 256, 8, 32
FO = C * C  # per-o mat size: 1024
N_CORES = 8
# phase-1/chain segments (start_agent, n_agents): small at the head so the
# PE starts early, small at the tail so the final chain steps hand off at
# fine granularity, big in the middle for DMA efficiency.
SEGS = [(4 * b, 4) for b in range(14)] + [(56, 2), (58, 2), (60, 2), (62, 2)]

_CACHE = {}


def _build():
    from concourse import bacc, mybir, tile
    from concourse.masks import make_identity
    from concourse.tile_rust import add_dep_helper

    F32 = mybir.dt.float32
    BF16 = mybir.dt.bfloat16
    FP8 = mybir.dt.float8e3
    SIG = mybir.ActivationFunctionType.Sigmoid
    COPY = mybir.ActivationFunctionType.Copy
    nc = bacc.Bacc(
        "TRN2", target_bir_lowering=False, debug=False, num_devices=N_CORES
    )
    x_d = nc.dram_tensor("inputs", [2, 128, A], FP8, kind="ExternalInput")
    mps_d = nc.dram_tensor(
        "mps", [2, 128, A * FO], FP8, kind="ExternalInput"
    )
    out_d = nc.dram_tensor("out", [C, C], BF16, kind="ExternalOutput")

    with tile.TileContext(nc) as tc:
        with (
            tc.tile_pool(name="mps_pool", bufs=9) as mps_pool,
            tc.tile_pool(name="small", bufs=1) as small,
            tc.tile_pool(name="cw_pool", bufs=6) as cw_pool,
            tc.tile_pool(name="env_pool", bufs=4) as env_pool,
            tc.tile_pool(name="ps_mat", bufs=4, space="PSUM") as ps_mat,
            tc.tile_pool(name="ps_chain", bufs=3, space="PSUM") as ps_chain,
        ):
            seg_of = {}
            for si, (g0, w) in enumerate(SEGS):
                for g in range(g0, g0 + w):
                    seg_of[g] = si

            # x_sb[e_lo, eh*64 + g] = x[g, eh*128 + e_lo]
            x_sb = small.tile([128, 2 * A], FP8)
            for eh in range(2):
                nc.sync.dma_start(x_sb[:, eh * A : (eh + 1) * A], x_d[eh])

            ident = small.tile([C, C], BF16)
            make_identity(nc, ident[:])

            envs = [None]
            cvs = {}

            def chain_step(g, after=None):
                si = seg_of[g]
                cv, gi = cvs[si], g - SEGS[si][0]
                init = g == 0
                ps_g = ps_chain.tile([C, C], BF16 if init else F32, tag="cps")
                if init:
                    pe = nc.tensor.transpose(ps_g[:], cv[:, gi, :], ident[:])
                else:
                    pe = nc.tensor.matmul(
                        ps_g[:],
                        cv[:, gi, :],
                        envs[0][:],
                        start=True,
                        stop=True,
                    )
                if after is not None:
                    # pin the chain matmul after the current agent block so
                    # the scheduler cannot bunch chain steps early either
                    add_dep_helper(
                        pe.ins,
                        after.ins,
                        sync=True,
                        reason="pace chain step to its agent slot",
                    )
                env2 = env_pool.tile([C, C], BF16, tag="env")
                if init:
                    # cw holds 256*mat (fp8 input scales 128*2); descale
                    nc.scalar.activation(
                        env2[:], ps_g[:], COPY, scale=1.0 / 256.0
                    )
                else:
                    # psum = env @ (256*mat); sigmoid's scale descales it
                    nc.scalar.activation(
                        env2[:], ps_g[:], SIG, scale=1.0 / 256.0
                    )
                envs[0] = env2
                return pe

            # Software-pipelined emission: the chain lags phase 1 by two
            # segments (so the DVE relayout of its weights has retired) and
            # is paced at one step per agent-slot (two when backlogged).
            # add_dep_helper pins each agent's phase-1 block AFTER the
            # latest chain matmul in the PE stream, so the Tile scheduler
            # cannot batch a segment's matmuls ahead of the chain — the
            # previous sigmoid retires while the phase-1 block runs and the
            # chain never head-of-line-blocks the PE.
            next_chain = 0
            last_chain = [None]

            for si, (g0, w) in enumerate(SEGS):
                psum_b = ps_mat.tile([128, 8 * w], F32, tag="psa")
                tvs = []
                for eh in range(2):
                    t = mps_pool.tile([128, w * FO], FP8, tag="mps")
                    nc.sync.dma_start(
                        t[:], mps_d[eh, :, g0 * FO : (g0 + w) * FO]
                    )
                    tvs.append(
                        t[:].rearrange(
                            "e (gi c p) -> e gi c p", gi=w, c=8, p=128
                        )
                    )
                for gi in range(w):
                    g = g0 + gi
                    last_mm = None
                    for c in range(8):
                        for eh in range(2):
                            mm = nc.tensor.matmul(
                                psum_b[:, gi * 8 + c : gi * 8 + c + 1],
                                tvs[eh][:, gi, c, :],
                                x_sb[:, eh * A + g : eh * A + g + 1],
                                start=(eh == 0),
                                stop=(eh == 1),
                            )
                            last_mm = mm
                            if last_chain[0] is not None:
                                # pin EVERY phase-1 matmul after the latest
                                # chain matmul (sync=True: no_sync edges
                                # are ignored for ordering) so phase-1
                                # cannot run ahead of the chain; with the
                                # reverse pin in chain_step the PE stream
                                # strictly alternates [agent block][chain
                                # step] and each sigmoid retires while the
                                # next agent block streams
                                add_dep_helper(
                                    mm.ins,
                                    last_chain[0].ins,
                                    sync=True,
                                    reason="interleave chain with phase-1",
                                )
                    # agents eligible for the chain: segments <= si - 2
                    elig = SEGS[si - 1][0] if si >= 1 else 0
                    backlog = elig - next_chain
                    cap = 2 if backlog > 4 else 1
                    k = 0
                    while next_chain < elig and k < cap:
                        last_chain[0] = chain_step(next_chain, after=last_mm)
                        next_chain += 1
                        k += 1
                # psum[32*jh + k, gi*8 + c] -> cw[k, gi*32 + jh*8 + c]:
                # one cross-quadrant DVE copy per psum quadrant jh.
                cw = cw_pool.tile([C, 32 * w], BF16, tag="cw", name=f"cw{si}")
                cwv = cw[:].rearrange(
                    "k (gi jh c) -> k gi jh c", gi=w, jh=4, c=8
                )
                for jh in range(4):
                    nc.vector.tensor_copy(
                        cwv[:, :, jh, :],
                        psum_b[32 * jh : 32 * jh + 32, :].rearrange(
                            "k (gi c) -> k gi c", gi=w, c=8
                        ),
                    )
                cvs[si] = cw[:].rearrange("k (gi j) -> k gi j", gi=w, j=32)
            while next_chain < A:
                chain_step(next_chain)
                next_chain += 1

            # ship the final 32x32 env; host takes the trace
            nc.sync.dma_start(out_d[:], envs[0][:])

    nc.compile()
    return nc


def get_nc():
    if "nc" not in _CACHE:
        _CACHE["nc"] = _build()
    return _CACHE["nc"]


def make_in_maps(inputs, mps):
    import ml_dtypes

    FP8 = ml_dtypes.float8_e3m4
    # power-of-two scales put both operands mid-range in e3m4 (max 15.5):
    # x ~ N(0,1) * 2, mps ~ N(0, 0.0156^2) * 128.  Combined 256x descales
    # on-device via the chain sigmoid's scale parameter.
    x = (np.asarray(inputs, dtype=np.float32) * 2.0).astype(FP8)
    mps = np.asarray(mps, dtype=np.float32).reshape(A, E, O, FO)
    # x packed as [e_chunk, e_low, agent]
    x_pack = np.ascontiguousarray(x.reshape(A, 2, 128).transpose(1, 2, 0))
    # F_idx[c, p] = k*32 + j with k = p%32, j = 8*(p//32) + c: weight column
    # (gi, c) reads the contiguous run packed[gi*1024 + c*128 : +128] and
    # psum partitions come out as 32*jh + k (chain-quadrant layout).
    p = np.arange(128)
    c = np.arange(8)[:, None]
    F_idx = ((p % 32) * 32 + (p // 32) * 8 + c).reshape(-1)  # (1024,)
    in_maps = []
    for ci in range(N_CORES):
        m = (mps[:, :, ci, :] * 128.0).astype(FP8)  # (A, E, FO)
        m = m[:, :, F_idx]  # permute f so weight columns are contiguous
        m = m.reshape(A, 2, 128, FO).transpose(1, 2, 0, 3)  # (2, 128, A, FO)
        in_maps.append(
            {
                "inputs": x_pack,
                "mps": np.ascontiguousarray(m).reshape(2, 128, A * FO),
            }
        )
    return in_maps


def kernel(inputs, mps):
    from concourse.bass_utils import run_bass_kernel_spmd

    nc = get_nc()
    in_maps = make_in_maps(inputs, mps)
    try:
        res = run_bass_kernel_spmd(nc, in_maps, core_ids=list(range(N_CORES)))
    except Exception:
        # rare transient NRT failures; one retry
        res = run_bass_kernel_spmd(nc, in_maps, core_ids=list(range(N_CORES)))
    return np.array(
        [
            np.trace(res.results[ci]["out"].astype(np.float32))
            for ci in range(N_CORES)
        ],
        dtype=np.float32,
    )


# revision 20
# speedup vs baseline: 1.3822x; 1.0201x over previous
"""Trainium2 (8-core SPMD) kernel for the ActorCriticTensorNet MPS head.

reference:
    env0 = einsum('e,eoij->oij', x[0], mps[0])
    for a in 1..63: env = sigmoid(env @ einsum('e,eoij->oij', x[a], mps[a]))
    out = einsum('oii->o', env)

Strategy: the computation factorizes perfectly over the output channel o —
the per-agent contractions mat[a][o] and the 63-step sigmoid chain for
channel o never touch any other channel; the channels only meet in the final
trace vector.  So shard by o: core c receives mps[:, :, c, :, :] (16.8 MB in
fp8) plus the full (tiny) x, computes all 64 mat[g][c] slices locally, runs
its own o=c chain locally, and ships its final 32x32 env; the host takes the
8 traces.  Zero inter-core communication.

Phase-1 layout (per agent): weight column (gi, eh, c) is the contiguous run
packed[(gi*2 + eh)*1024 + c*128 : +128] with per-column packing
p = 32*jh + k (k = chain row, jh = j>>3, c = j&7), so
psum[32*jh + k, gi*8 + c] = mat[gi][k, 8*jh + c] after the two-eh
accumulation.  Four 32-partition DVE copies (cross-quadrant moves are free)
then drop each psum quadrant jh straight into the chain-weight slab
cw[k, gi*32 + 8*jh + c] = mat[gi][k, j] in SBUF — no DRAM round trip.

Phase-1 operands are fp8 e3m4 with power-of-two scales (mps x128, x x2 —
both mid-range in e3m4's +-15.5), halving HBM traffic vs bf16 and enabling
4x fast-weight-load (LDWEIGHTS ~27 ns per 128x128 block).  PSUM accumulates
the 256x-scaled mats in fp32; the 1/256 descale folds into the chain
sigmoid's scale parameter.  The chain runs in bf16 (simulated end-to-end
relative error ~1.4e-3; gate is 2e-2).

Scheduling: the 63-step sigmoid chain is strictly serial at ~650 ns/step
(MM 200 + sem + ACT 277 + sem), so it must fully overlap phase 1.  The Tile
scheduler ignores program order, so the PE stream is pinned with
add_dep_helper(sync=True) edges into strict alternation
[16-MM agent block][one chain step]: each sigmoid retires while the next
agent block streams, and the whole kernel runs at the chain's serial rate
with phase 1 and the DMA stream hidden beneath it.  One contiguous DMA per
segment (both e-halves packed together host-side) keeps the HWDGE dispatch
cost (~650 ns each) off the critical path; the chain lags two segments
(one for the tail segments) so its weights are always relayouted in time.
"""

import numpy as np

A, E, O, C = 64, 256, 8, 32
FO = C * C  # per-o mat size: 1024
N_CORES = 8
SEGS = [(4 * b, 4) for b in range(14)] + [(56, 2), (58, 2), (60, 2), (62, 2)]
TAIL_SEGS = 3  # segments that run with one-segment chain lag

_CACHE = {}


def _build():
    from concourse import bacc, mybir, tile
    from concourse.masks import make_identity
    from concourse.tile_rust import add_dep_helper

    F32 = mybir.dt.float32
    BF16 = mybir.dt.bfloat16
    FP8 = mybir.dt.float8e3
    SIG = mybir.ActivationFunctionType.Sigmoid
    COPY = mybir.ActivationFunctionType.Copy
    nc = bacc.Bacc(
        "TRN2", target_bir_lowering=False, debug=False, num_devices=N_CORES
    )
    x_d = nc.dram_tensor("inputs", [128, 2 * A], FP8, kind="ExternalInput")
    mps_d = nc.dram_tensor(
        "mps", [128, A * 2 * FO], FP8, kind="ExternalInput"
    )
    out_d = nc.dram_tensor("out", [C, C], BF16, kind="ExternalOutput")

    with tile.TileContext(nc) as tc:
        with (
            tc.tile_pool(name="mps_pool", bufs=8) as mps_pool,
            tc.tile_pool(name="small", bufs=1) as small,
            tc.tile_pool(name="cw_pool", bufs=6) as cw_pool,
            tc.tile_pool(name="env_pool", bufs=4) as env_pool,
            tc.tile_pool(name="ps_mat", bufs=4, space="PSUM") as ps_mat,
            tc.tile_pool(name="ps_chain", bufs=3, space="PSUM") as ps_chain,
        ):
            seg_of = {}
            for si, (g0, w) in enumerate(SEGS):
                for g in range(g0, g0 + w):
                    seg_of[g] = si

            # all input DMAs dispatch before anything else touches the
            # sync queue so the stream ramps as early as possible
            x_sb = small.tile([128, 2 * A], FP8)
            nc.sync.dma_start(x_sb[:], x_d[:])
            seg_tiles = []
            for si, (g0, w) in enumerate(SEGS):
                t = mps_pool.tile([128, w * 2 * FO], FP8, tag="mps")
                nc.sync.dma_start(
                    t[:], mps_d[:, g0 * 2 * FO : (g0 + w) * 2 * FO]
                )
                seg_tiles.append(
                    t[:].rearrange(
                        "e (gi eh c p) -> e gi eh c p", gi=w, eh=2, c=8, p=128
                    )
                )

            ident = small.tile([C, C], BF16)
            make_identity(nc, ident[:])

            envs = [None]
            cvs = {}

            def chain_step(g, after=None):
                si = seg_of[g]
                cv, gi = cvs[si], g - SEGS[si][0]
                init = g == 0
                ps_g = ps_chain.tile([C, C], BF16 if init else F32, tag="cps")
                if init:
                    pe = nc.tensor.transpose(ps_g[:], cv[:, gi, :], ident[:])
                else:
                    pe = nc.tensor.matmul(
                        ps_g[:],
                        cv[:, gi, :],
                        envs[0][:],
                        start=True,
                        stop=True,
                    )
                if after is not None:
                    # pin the chain matmul after the current agent block so
                    # the scheduler cannot bunch chain steps early either
                    add_dep_helper(
                        pe.ins,
                        after.ins,
                        sync=True,
                        reason="pace chain step to its agent slot",
                    )
                env2 = env_pool.tile([C, C], BF16, tag="env")
                if init:
                    # cw holds 256*mat (fp8 input scales 128*2); descale
                    nc.scalar.activation(
                        env2[:], ps_g[:], COPY, scale=1.0 / 256.0
                    )
                else:
                    # psum = env @ (256*mat); sigmoid's scale descales it
                    nc.scalar.activation(
                        env2[:], ps_g[:], SIG, scale=1.0 / 256.0
                    )
                envs[0] = env2
                return pe

            next_chain = 0
            last_chain = [None]

            for si, (g0, w) in enumerate(SEGS):
                psum_b = ps_mat.tile([128, 8 * w], F32, tag="psa")
                tvs = seg_tiles[si]
                for gi in range(w):
                    g = g0 + gi
                    last_mm = None
                    for c in range(8):
                        for eh in range(2):
                            mm = nc.tensor.matmul(
                                psum_b[:, gi * 8 + c : gi * 8 + c + 1],
                                tvs[:, gi, eh, c, :],
                                x_sb[:, eh * A + g : eh * A + g + 1],
                                start=(eh == 0),
                                stop=(eh == 1),
                            )
                            last_mm = mm
                            if last_chain[0] is not None:
                                # pin EVERY phase-1 matmul after the latest
                                # chain matmul (sync=True: no_sync edges
                                # are ignored for ordering) so phase-1
                                # cannot run ahead of the chain; with the
                                # reverse pin in chain_step the PE stream
                                # strictly alternates [agent block][chain
                                # step] and each sigmoid retires while the
                                # next agent block streams
                                add_dep_helper(
                                    mm.ins,
                                    last_chain[0].ins,
                                    sync=True,
                                    reason="interleave chain with phase-1",
                                )
                    # chain eligibility: two-segment lag (one for the tail
                    # segments, where phase-1 is ending and a short stall
                    # on the relayout costs nothing)
                    ei = si if si >= len(SEGS) - TAIL_SEGS else si - 1
                    elig = SEGS[ei][0] if ei >= 0 else 0
                    backlog = elig - next_chain
                    cap = 2 if backlog > 4 else 1
                    k = 0
                    while next_chain < elig and k < cap:
                        last_chain[0] = chain_step(next_chain, after=last_mm)
                        next_chain += 1
                        k += 1
                # psum[32*jh + k, gi*8 + c] -> cw[k, gi*32 + jh*8 + c]:
                # one cross-quadrant DVE copy per psum quadrant jh.
                cw = cw_pool.tile([C, 32 * w], BF16, tag="cw", name=f"cw{si}")
                cwv = cw[:].rearrange(
                    "k (gi jh c) -> k gi jh c", gi=w, jh=4, c=8
                )
                for jh in range(4):
                    nc.vector.tensor_copy(
                        cwv[:, :, jh, :],
                        psum_b[32 * jh : 32 * jh + 32, :].rearrange(
                            "k (gi c) -> k gi c", gi=w, c=8
                        ),
                    )
                cvs[si] = cw[:].rearrange("k (gi j) -> k gi j", gi=w, j=32)
            while next_chain < A:
                chain_step(next_chain)
                next_chain += 1

            # ship the final 32x32 env; host takes the trace
            nc.sync.dma_start(out_d[:], envs[0][:])

    nc.compile()
    return nc


def get_nc():
    if "nc" not in _CACHE:
        _CACHE["nc"] = _build()
    return _CACHE["nc"]


def make_in_maps(inputs, mps):
    import ml_dtypes

    FP8 = ml_dtypes.float8_e3m4
    # power-of-two scales put both operands mid-range in e3m4 (max 15.5):
    # x ~ N(0,1) * 2, mps ~ N(0, 0.0156^2) * 128.  Combined 256x descales
    # on-device via the chain sigmoid's scale parameter.
    x = (np.asarray(inputs, dtype=np.float32) * 2.0).astype(FP8)
    mps = np.asarray(mps, dtype=np.float32).reshape(A, E, O, FO)
    # x packed as [e_low, (e_chunk, agent)]
    x_pack = np.ascontiguousarray(
        x.reshape(A, 2, 128).transpose(2, 1, 0).reshape(128, 2 * A)
    )
    # F_idx[c, p] = k*32 + j with k = p%32, j = 8*(p//32) + c: weight column
    # (gi, eh, c) reads the contiguous run packed[(gi*2+eh)*1024 + c*128 :
    # +128] and psum partitions come out as 32*jh + k (chain-quadrant
    # layout).
    p = np.arange(128)
    c = np.arange(8)[:, None]
    F_idx = ((p % 32) * 32 + (p // 32) * 8 + c).reshape(-1)  # (1024,)
    in_maps = []
    for ci in range(N_CORES):
        m = (mps[:, :, ci, :] * 128.0).astype(FP8)  # (A, E, FO)
        m = m[:, :, F_idx]  # permute f so weight columns are contiguous
        # -> [e_low(128), agent, e_chunk, FO]  (one contiguous run per
        #    segment: both e-halves of its agents)
        m = m.reshape(A, 2, 128, FO).transpose(2, 0, 1, 3)
        in_maps.append(
            {
                "inputs": x_pack,
                "mps": np.ascontiguousarray(m).reshape(128, A * 2 * FO),
            }
        )
    return in_maps


def kernel(inputs, mps):
    from concourse.bass_utils import run_bass_kernel_spmd

    nc = get_nc()
    in_maps = make_in_maps(inputs, mps)
    try:
        res = run_bass_kernel_spmd(nc, in_maps, core_ids=list(range(N_CORES)))
    except Exception:
        # rare transient NRT failures; one retry
        res = run_bass_kernel_spmd(nc, in_maps, core_ids=list(range(N_CORES)))
    return np.array(
        [
            np.trace(res.results[ci]["out"].astype(np.float32))
            for ci in range(N_CORES)
        ],
        dtype=np.float32,
    )
